# revision 1
# baseline (speedup 1.0000x reference)
"""Trainium2 Bass kernel for CompanyIndustryAttention (gnn_message_passing).

Strategy (all 8 cores, zero collectives):
  - Companies sharded into 8 contiguous ranges of 2500 rows; each edge is
    owned by the core that owns its src company, so the segment-sum scatter
    is core-local (no all-reduce needed).
  - K/V side: tgt indexes only 500 industries, so softmax over the full
    edge set collapses to a count-weighted softmax over the 500 industries:
        sum_k exp(s_tgt[k]) v_tgt[k] = sum_u cnt_u exp(s_u) v_u
    implemented exactly by appending ln(cnt_u) as a 65th feature row on the
    K side (exp(s + ln c) = c * exp(s)).  This turns O(E x E) attention into
    O(E x 500).
  - Device work is fully dense/static: host does index-only preprocessing
    (sort edges by src, pack into per-company-tile slot windows, gather
    company_x rows for the Q side, count edges).  The compiled program is
    identical on all cores; per-core differences live in the input tensors.
  - Segment-sum on device = one-hot(src) matmuls on the tensor engine over
    a fixed 2-e-tile window per company tile (host packing guarantees the
    window); layernorm tail runs node-major with bn_stats/bn_aggr.
"""

import os
import sys

import numpy as np

for _p in ("/opt/trn_rl_repo",):
    if _p not in sys.path and os.path.isdir(_p):
        sys.path.insert(0, _p)

import concourse.bass as bass
import concourse.bacc as bacc
import concourse.tile as tile
from concourse import mybir
from concourse.bass_utils import run_bass_kernel_spmd

F32 = mybir.dt.float32
AF = mybir.ActivationFunctionType
ALU = mybir.AluOpType

# Problem shapes (hardcoded per the spec).
N_COMPANY, N_INDUSTRY, E = 20000, 500, 8192
CC, CI, D, H = 256, 128, 256, 4
HD = D // H  # 64
SCALE = 1.0 / float(np.sqrt(np.float32(HD)))

NCORES = 8
NSH = N_COMPANY // NCORES       # 2500 companies per core
NCT = 20                        # company tiles (19 x 128 + 68)
E_CAP = 1280                    # padded edge slots per core (10 e-tiles)
NET = E_CAP // 128              # 10 edge tiles
SLOTS = E_CAP // NCT            # 64 slots per company tile
E_CHUNKS = [(0, 512), (512, 1024), (1024, 1280)]

_CACHE = {}
TRACE = False        # set by test.py to request an NTFF profile
LAST_RESULT = None   # BassKernelResults of the most recent run


def _csz(j):
    return min(128, NSH - 128 * j)


def _window(j):
    return [t for t in (j // 2, j // 2 + 1) if t < NET]


def build_program():
    nc = bacc.Bacc(debug=False)

    # ---- I/O declarations (per-core tensors; same names on every core) ----
    def din(name, shape):
        return nc.declare_dram_parameter(name, list(shape), F32, isOutput=False)

    cxT = din("cxT", (CC, NSH))          # company_x shard, transposed
    qxT = din("qxT", (CC, E_CAP))        # company_x rows gathered per edge slot
    ixT = din("ixT", (CI, N_INDUSTRY))   # industry_x transposed
    WcT = din("WcT", (CC, D))
    WiT = din("WiT", (CI, D))
    wqT = din("wqT", (D, D))             # (wq*scale).T
    wkT = din("wkT", (D, D))
    wvT = din("wvT", (D, D))
    woT = din("woT", (D, D))             # w_out.T
    bc = din("bc", (1, D))
    bi = din("bi", (1, D))
    bq = din("bq", (1, D))               # bq*scale
    bk = din("bk", (1, D))
    bv = din("bv", (1, D))
    bo = din("bo", (1, D))
    gamma = din("gamma", (1, D))
    beta = din("beta", (1, D))
    lncnt = din("lncnt", (1, N_INDUSTRY))   # ln(edge count per industry)
    srcf = din("srcf", (E_CAP,))            # local src id per slot (-1 = pad)
    recip = din("recip", (2560,))           # 1/(cnt_company+1e-6), padded
    iotac = din("iotac", (1, NSH))          # 0..2499
    out = nc.declare_dram_parameter("out", [NSH, D], F32, isOutput=True)

    def wrap_ap(t, n_elems, cols):
        # [n] DRAM -> [128, cols] SBUF with element (p + 128*c) at [p, c]
        return bass.AP(tensor=t[:].tensor, offset=0, ap=[[1, 128], [128, cols]])

    with tile.TileContext(nc) as tc:
        with (
            tc.tile_pool(name="const", bufs=1) as const,
            tc.tile_pool(name="persist", bufs=1) as persist,
            tc.tile_pool(name="work", bufs=3) as work,
            tc.tile_pool(name="ohp", bufs=4) as ohp,
            # PSUM budget (16KB/partition, bank=2KB): ps x2 + pc x2 + pb,
            # pagg, pch x1 = 14KB
            tc.tile_pool(name="psA", bufs=2, space="PSUM") as psum_a,
            tc.tile_pool(name="psB", bufs=1, space="PSUM") as psum_b,
        ):
            dma = nc.sync.dma_start

            # ---------------- constants / params into SBUF ----------------
            def load2(t, rows, cols):
                # [rows, cols] DRAM (rows multiple of 128) -> list of [128, cols]
                tiles = []
                for k in range(rows // 128):
                    s = const.tile([128, cols], F32, name=f"w_{t.name}_{k}", tag=f"w_{t.name}_{k}")
                    dma(out=s, in_=t[k * 128:(k + 1) * 128, :])
                    tiles.append(s)
                return tiles

            cxT_sb = load2(cxT, CC, NSH)
            qxT_sb = load2(qxT, CC, E_CAP)
            ixT_sb = load2(ixT, CI, N_INDUSTRY)
            WcT_sb = load2(WcT, CC, D)
            WiT_sb = load2(WiT, CI, D)
            wqT_sb = load2(wqT, D, D)
            wkT_sb = load2(wkT, D, D)
            wvT_sb = load2(wvT, D, D)
            woT_sb = load2(woT, D, D)

            def bcast_row(t, tag):
                s = const.tile([128, D], F32, tag=tag)
                dma(out=s, in_=t[:, :].to_broadcast([128, D]))
                return s

            bc_b = bcast_row(bc, "bc_b")
            bv_b = bcast_row(bv, "bv_b")
            bo_b = bcast_row(bo, "bo_b")
            gam_b = bcast_row(gamma, "gam_b")
            bet_b = bcast_row(beta, "bet_b")

            def col_pp(t, tag):
                # [1, 256] DRAM -> [128, 2] SBUF per-partition columns
                s = const.tile([128, 2], F32, tag=tag)
                dma(out=s, in_=bass.AP(tensor=t[:, :].tensor, offset=0,
                                       ap=[[1, 128], [128, 2]]))
                return s

            bc_pp = col_pp(bc, "bc_pp")
            bi_pp = col_pp(bi, "bi_pp")
            bq_pp = col_pp(bq, "bq_pp")
            bk_pp = col_pp(bk, "bk_pp")

            iota_b = const.tile([128, NSH], F32, name="iota_b", tag="iota_b")
            dma(out=iota_b, in_=iotac[:, :].to_broadcast([128, NSH]))

            src_sb = const.tile([128, NET], F32, name="src_sb", tag="src_sb")
            dma(out=src_sb, in_=wrap_ap(srcf, E_CAP, NET))
            recip_sb = const.tile([128, NCT], F32, name="recip_sb", tag="recip_sb")
            dma(out=recip_sb, in_=wrap_ap(recip, 2560, NCT))

            ones64 = const.tile([1, HD], F32, name="ones64", tag="ones64")
            nc.vector.memset(ones64, 1.0)
            eps_sb = const.tile([128, 1], F32, name="eps_sb", tag="eps_sb")
            nc.vector.memset(eps_sb, 1e-5)

            def ppbias(colsb, h):
                # per-partition bias [64,1] for head h from a [128,2] column tile
                return colsb[64 * (h % 2):64 * (h % 2) + 64, h // 2:h // 2 + 1]

            # ---------------- industry side: ihT, kh', v' -------------------
            # industry_hT [D, 500] feature-major
            ihT = [persist.tile([128, N_INDUSTRY], F32, name=f"ihT{d}", tag=f"ihT{d}")
                   for d in range(2)]
            for dti in range(2):
                ps = psum_a.tile([128, 512], F32, name="ps", tag="ps")
                nc.tensor.matmul(ps[:, 0:N_INDUSTRY],
                                 WiT_sb[0][:, dti * 128:(dti + 1) * 128],
                                 ixT_sb[0], start=True, stop=True)
                nc.scalar.activation(ihT[dti], ps[:, 0:N_INDUSTRY], AF.Identity,
                                     bias=bi_pp[:, dti:dti + 1], scale=1.0)

            # kh' per head: [65, 500]; row 64 = ln(cnt)
            khp = [persist.tile([128, N_INDUSTRY], F32, name=f"khp{h}", tag=f"khp{h}")
                   for h in range(H)]
            for h in range(H):
                ps = psum_a.tile([128, 512], F32, name="ps", tag="ps")
                for k in range(2):
                    nc.tensor.matmul(ps[0:64, 0:N_INDUSTRY],
                                     wkT_sb[k][:, h * 64:(h + 1) * 64],
                                     ihT[k], start=(k == 0), stop=(k == 1))
                nc.scalar.activation(khp[h][0:64, :], ps[0:64, 0:N_INDUSTRY],
                                     AF.Identity,
                                     bias=ppbias(bk_pp, h), scale=1.0)
                dma(out=khp[h][64:65, :], in_=lncnt[:, :])

            # v' node-major [500-part, H, 65]; col 64 of each head = 1.0
            usz = [128, 128, 128, 116]
            vp = [persist.tile([128, H, HD + 1], F32, name=f"vp{t}", tag=f"vp{t}")
                  for t in range(4)]
            for t in range(4):
                u0, u1 = t * 128, t * 128 + usz[t]
                ps = psum_a.tile([128, 512], F32, name="ps", tag="ps")
                for k in range(2):
                    nc.tensor.matmul(ps[0:usz[t], 0:D],
                                     ihT[k][:, u0:u1], wvT_sb[k],
                                     start=(k == 0), stop=(k == 1))
                for h in range(H):
                    nc.vector.tensor_tensor(
                        out=vp[t][0:usz[t], h, 0:HD],
                        in0=ps[0:usz[t], h * 64:(h + 1) * 64],
                        in1=bv_b[0:usz[t], h * 64:(h + 1) * 64],
                        op=ALU.add)
                nc.vector.memset(vp[t][:, :, HD:HD + 1], 1.0)

            # ---------------- q side: q_h then qh' --------------------------
            # q_hT [D, E_CAP] = Wc @ qxT + bc   (feature-major)
            qhT = [persist.tile([128, E_CAP], F32, name=f"qhT{d}", tag=f"qhT{d}")
                   for d in range(2)]
            for dti in range(2):
                for c0, c1 in E_CHUNKS:
                    ps = psum_a.tile([128, 512], F32, name="ps", tag="ps")
                    for k in range(2):
                        nc.tensor.matmul(
                            ps[:, 0:c1 - c0],
                            WcT_sb[k][:, dti * 128:(dti + 1) * 128],
                            qxT_sb[k][:, c0:c1],
                            start=(k == 0), stop=(k == 1))
                    nc.scalar.activation(qhT[dti][:, c0:c1], ps[:, 0:c1 - c0],
                                         AF.Identity,
                                         bias=bc_pp[:, dti:dti + 1], scale=1.0)

            # qh' per head [65, E_CAP] (scaled); row 64 = 1.0
            qhp = [persist.tile([128, E_CAP], F32, name=f"qhp{h}", tag=f"qhp{h}")
                   for h in range(H)]
            for h in range(H):
                for c0, c1 in E_CHUNKS:
                    ps = psum_a.tile([128, 512], F32, name="ps", tag="ps")
                    for k in range(2):
                        nc.tensor.matmul(ps[0:64, 0:c1 - c0],
                                         wqT_sb[k][:, h * 64:(h + 1) * 64],
                                         qhT[k][:, c0:c1],
                                         start=(k == 0), stop=(k == 1))
                    nc.scalar.activation(qhp[h][0:64, c0:c1],
                                         ps[0:64, 0:c1 - c0], AF.Identity,
                                         bias=ppbias(bq_pp, h), scale=1.0)
                nc.vector.memset(qhp[h][64:65, :], 1.0)

            # ---------------- attention: scores -> exp -> ctx ---------------
            # ctxT [D, E_CAP] feature-major (normalized per head)
            ctxT = [persist.tile([128, E_CAP], F32, name=f"ctxT{d}", tag=f"ctxT{d}")
                    for d in range(2)]
            for h in range(H):
                for c0, c1 in E_CHUNKS:
                    cw = c1 - c0
                    pc = psum_a.tile([128, 512], F32, name="pc", tag="pc")
                    for t in range(4):
                        u0, u1 = t * 128, t * 128 + usz[t]
                        ps = psum_a.tile([128, 512], F32, name="ps", tag="ps")
                        nc.tensor.matmul(ps[0:usz[t], 0:cw],
                                         khp[h][0:65, u0:u1],
                                         qhp[h][0:65, c0:c1],
                                         start=True, stop=True)
                        pexp = work.tile([128, 512], F32, name="pexp", tag="pexp")
                        nc.scalar.activation(pexp[0:usz[t], 0:cw],
                                             ps[0:usz[t], 0:cw], AF.Exp)
                        nc.tensor.matmul(pc[0:65, 0:cw],
                                         vp[t][0:usz[t], h, :],
                                         pexp[0:usz[t], 0:cw],
                                         start=(t == 0), stop=(t == 3))
                    # normalize: rows 0:64 / row 64
                    rd = work.tile([1, 512], F32, name="rd", tag="rd")
                    nc.vector.reciprocal(rd[:, 0:cw], pc[64:65, 0:cw])
                    pb = psum_b.tile([128, 512], F32, name="pb", tag="pb")
                    nc.tensor.matmul(pb[0:64, 0:cw], ones64, rd[:, 0:cw],
                                     start=True, stop=True)
                    rdb = work.tile([128, 512], F32, name="rdb", tag="rdb")
                    nc.scalar.activation(rdb[0:64, 0:cw], pb[0:64, 0:cw],
                                         AF.Copy)
                    nc.vector.tensor_tensor(
                        out=ctxT[h // 2][64 * (h % 2):64 * (h % 2) + 64, c0:c1],
                        in0=pc[0:64, 0:cw], in1=rdb[0:64, 0:cw], op=ALU.mult)

            # ---------------- attn_out (node-major) -------------------------
            ao = [persist.tile([128, D], F32, name=f"ao{t}", tag=f"ao{t}") for t in range(NET)]
            for t in range(NET):
                ps = psum_a.tile([128, 512], F32, name="ps", tag="ps")
                for k in range(2):
                    nc.tensor.matmul(ps[:, 0:D],
                                     ctxT[k][:, t * 128:(t + 1) * 128],
                                     woT_sb[k], start=(k == 0), stop=(k == 1))
                nc.vector.tensor_tensor(out=ao[t], in0=ps[:, 0:D], in1=bo_b,
                                        op=ALU.add)

            # ------------- segment sum + residual + layernorm ---------------
            for j in range(NCT):
                cs = _csz(j)
                pagg = psum_b.tile([128, D], F32, name="pagg", tag="pagg")
                win = _window(j)
                for wi, t in enumerate(win):
                    oh = ohp.tile([128, 128], F32, name="oh", tag="oh")
                    nc.vector.tensor_tensor(
                        out=oh[:, 0:cs],
                        in0=src_sb[:, t:t + 1].to_broadcast([128, cs]),
                        in1=iota_b[:, 128 * j:128 * j + cs],
                        op=ALU.is_equal)
                    nc.tensor.matmul(pagg[0:cs, :], oh[:, 0:cs], ao[t],
                                     start=(wi == 0), stop=(wi == len(win) - 1))
                # company_h for this tile
                pch = psum_b.tile([128, D], F32, name="pch", tag="pch")
                for k in range(2):
                    nc.tensor.matmul(pch[0:cs, :],
                                     cxT_sb[k][:, 128 * j:128 * j + cs],
                                     WcT_sb[k], start=(k == 0), stop=(k == 1))
                ch = work.tile([128, D], F32, name="ch", tag="ch")
                nc.vector.tensor_tensor(out=ch[0:cs, :], in0=pch[0:cs, :],
                                        in1=bc_b[0:cs, :], op=ALU.add)
                # x = agg * recip + company_h
                x = work.tile([128, D], F32, name="x", tag="x")
                nc.vector.scalar_tensor_tensor(
                    out=x[0:cs, :], in0=pagg[0:cs, :],
                    scalar=recip_sb[0:cs, j:j + 1], in1=ch[0:cs, :],
                    op0=ALU.mult, op1=ALU.add)
                # layernorm along free axis
                st = work.tile([128, nc.vector.BN_STATS_DIM], F32, name="st", tag="st")
                nc.vector.bn_stats(out=st[0:cs, :], in_=x[0:cs, :])
                mv = work.tile([128, nc.vector.BN_AGGR_DIM], F32, name="mv", tag="mv")
                nc.vector.bn_aggr(out=mv[0:cs, :], in_=st[0:cs, :])
                sd = work.tile([128, 1], F32, name="sd", tag="sd")
                nc.scalar.activation(sd[0:cs, :], mv[0:cs, 1:2], AF.Sqrt,
                                     bias=eps_sb[0:cs, :], scale=1.0)
                rstd = work.tile([128, 1], F32, name="rstd", tag="rstd")
                nc.vector.reciprocal(rstd[0:cs, :], sd[0:cs, :])
                xn = work.tile([128, D], F32, name="xn", tag="xn")
                nc.vector.tensor_scalar(
                    out=xn[0:cs, :], in0=x[0:cs, :],
                    scalar1=mv[0:cs, 0:1], scalar2=rstd[0:cs, :],
                    op0=ALU.subtract, op1=ALU.mult)
                y = work.tile([128, D], F32, name="y", tag="y")
                nc.vector.tensor_tensor(out=y[0:cs, :], in0=xn[0:cs, :],
                                        in1=gam_b[0:cs, :], op=ALU.mult)
                nc.vector.tensor_tensor(out=y[0:cs, :], in0=y[0:cs, :],
                                        in1=bet_b[0:cs, :], op=ALU.add)
                dma(out=out[128 * j:128 * j + cs, :], in_=y[0:cs, :])

    if not nc.is_finalized():
        nc.finalize()   # Bacc: runs wait-splitting etc. to meet HW limits
    return nc


def _prep_core(core, company_x, edge_index, tgt_cnt):
    """Host-side index preprocessing for one core. Returns per-core arrays."""
    src = edge_index[0].astype(np.int64)
    lo = core * NSH
    sel = np.nonzero((src >= lo) & (src < lo + NSH))[0]
    ls = src[sel] - lo
    order = np.argsort(ls, kind="stable")
    ls = ls[order]
    gsel = sel[order]

    ctile = (ls // 128).astype(np.int64)
    cnts = np.bincount(ctile, minlength=NCT)

    slot_of = np.empty(len(ls), dtype=np.int64)
    s = 0
    pos = 0
    for j in range(NCT):
        s = max(SLOTS * j, s)
        e = s + cnts[j]
        if cnts[j] > 0:
            lo_t, hi_t = s // 128, (e - 1) // 128
            if not ({lo_t, hi_t} <= set(_window(j))) or e > E_CAP:
                return None  # packing violated -> caller falls back
            slot_of[pos:pos + cnts[j]] = np.arange(s, e)
            pos += cnts[j]
        s = e

    srcf = np.full(E_CAP, -1.0, dtype=np.float32)
    srcf[slot_of] = ls.astype(np.float32)
    qx = np.broadcast_to(company_x[lo], (E_CAP, CC)).copy()
    qx[slot_of] = company_x[lo + ls]

    ccnt = np.bincount(ls, minlength=NSH).astype(np.float32)
    recip = np.zeros(2560, dtype=np.float32)
    recip[:NSH] = np.float32(1.0) / (ccnt + np.float32(1e-6))

    return {
        "cxT": np.ascontiguousarray(company_x[lo:lo + NSH].T),
        "qxT": np.ascontiguousarray(qx.T),
        "srcf": srcf,
        "recip": recip,
    }


def _numpy_fallback(company_x, industry_x, edge_index, Wc, bc, Wi, bi,
                    w_in, b_in, w_out, b_out, gamma, beta):
    # Correctness safety net for inputs whose edge distribution breaks the
    # compiled packing assumptions. Mirrors the reference computation.
    company_h = company_x @ Wc.T + bc
    industry_h = industry_x @ Wi.T + bi
    src, tgt = edge_index[0], edge_index[1]
    e = src.shape[0]
    wq, wk, wv = np.split(w_in, 3, axis=0)
    bq, bk, bv = np.split(b_in, 3)
    qh = (company_h[src] @ wq.T + bq).reshape(e, H, HD)
    kh = (industry_h[tgt] @ wk.T + bk).reshape(e, H, HD)
    vh = (industry_h[tgt] @ wv.T + bv).reshape(e, H, HD)
    scores = np.einsum("qhd,khd->hqk", qh / np.sqrt(HD), kh)
    scores -= scores.max(-1, keepdims=True)
    p = np.exp(scores)
    attn = p / p.sum(-1, keepdims=True)
    ctx = np.einsum("hqk,khd->qhd", attn, vh).reshape(e, D)
    attn_out = ctx @ w_out.T + b_out
    agg = np.zeros((N_COMPANY, D), np.float32)
    np.add.at(agg, src, attn_out)
    counts = np.bincount(src, minlength=N_COMPANY).astype(np.float32)
    pooled = agg / (counts[:, None] + 1e-6)
    out = company_h + pooled
    mean = out.mean(-1, keepdims=True)
    var = out.var(-1, keepdims=True)
    return ((out - mean) / np.sqrt(var + 1e-5) * gamma + beta).astype(np.float32)


def kernel(company_x, industry_x, edge_index, Wc, bc, Wi, bi,
           w_in, b_in, w_out, b_out, gamma, beta):
    company_x = np.asarray(company_x, dtype=np.float32)
    industry_x = np.asarray(industry_x, dtype=np.float32)
    edge_index = np.asarray(edge_index)
    Wc = np.asarray(Wc, np.float32); bc = np.asarray(bc, np.float32)
    Wi = np.asarray(Wi, np.float32); bi = np.asarray(bi, np.float32)
    w_in = np.asarray(w_in, np.float32); b_in = np.asarray(b_in, np.float32)
    w_out = np.asarray(w_out, np.float32); b_out = np.asarray(b_out, np.float32)
    gamma = np.asarray(gamma, np.float32); beta = np.asarray(beta, np.float32)

    tgt = edge_index[1].astype(np.int64)
    tgt_cnt = np.bincount(tgt, minlength=N_INDUSTRY).astype(np.float32)

    cores = []
    for core in range(NCORES):
        pc = _prep_core(core, company_x, edge_index, tgt_cnt)
        if pc is None:
            print("kernel.py: edge packing fell outside compiled windows; "
                  "using host fallback", file=sys.stderr)
            return _numpy_fallback(company_x, industry_x, edge_index, Wc, bc,
                                   Wi, bi, w_in, b_in, w_out, b_out,
                                   gamma, beta)
        cores.append(pc)

    wq, wk, wv = np.split(w_in, 3, axis=0)
    bq, bk, bv = np.split(b_in, 3)
    with np.errstate(divide="ignore"):
        lncnt = np.log(tgt_cnt).astype(np.float32)

    shared = {
        "ixT": np.ascontiguousarray(industry_x.T),
        "WcT": np.ascontiguousarray(Wc.T),
        "WiT": np.ascontiguousarray(Wi.T),
        "wqT": np.ascontiguousarray((wq * np.float32(SCALE)).T),
        "wkT": np.ascontiguousarray(wk.T),
        "wvT": np.ascontiguousarray(wv.T),
        "woT": np.ascontiguousarray(w_out.T),
        "bc": bc.reshape(1, D), "bi": bi.reshape(1, D),
        "bq": (bq * np.float32(SCALE)).reshape(1, D),
        "bk": bk.reshape(1, D), "bv": bv.reshape(1, D),
        "bo": b_out.reshape(1, D),
        "gamma": gamma.reshape(1, D), "beta": beta.reshape(1, D),
        "lncnt": lncnt.reshape(1, N_INDUSTRY),
        "iotac": np.arange(NSH, dtype=np.float32).reshape(1, NSH),
    }

    if "nc" not in _CACHE:
        _CACHE["nc"] = build_program()
    nc = _CACHE["nc"]

    in_maps = [{**shared, **cores[i]} for i in range(NCORES)]
    kw = {}
    if TRACE:
        kw = {"trace": True, "tmpdir": os.environ.get("BASS_TRACE_DIR")}
    res = run_bass_kernel_spmd(nc, in_maps, list(range(NCORES)), **kw)
    global LAST_RESULT
    LAST_RESULT = res
    return np.concatenate([res.results[i]["out"] for i in range(NCORES)],
                          axis=0)



# revision 14
# speedup vs baseline: 2.2706x; 2.2706x over previous
"""Trainium2 Bass kernel for CompanyIndustryAttention (gnn_message_passing).

Strategy (all 8 cores, zero collectives):
  - Companies sharded into 8 contiguous ranges of 2500 rows; each edge is
    owned by the core that owns its src company, so the segment-sum scatter
    is core-local (no all-reduce needed).
  - K/V side: tgt indexes only 500 industries, so softmax over the full
    edge set collapses to a count-weighted softmax over the 500 industries
    (exp(s + ln c) = c * exp(s), with ln(c) applied as the per-partition
    activation bias of the Exp).  This turns O(E x E) attention into
    O(E x 500).
  - Q side dedup: edges sharing a src produce identical attention rows, and
    pooled = agg/(cnt+1e-6) ~= attn_out exactly, so only one slot per
    UNIQUE src company is computed (~870/core instead of ~1280 padded).
  - All matmuls run in bf16 (fp32 runs at 1/4 rate on the PE); fp32
    accumulation in PSUM.  Weight chains are pre-fused on the host
    (Wq_eff = wq@Wc etc.) to skip whole matmul stages.
  - The segment-sum one-hot matrices are precomputed on the host and
    DMA'd in as bf16, feeding plain matmuls.
  - Layernorm tail is spread across engines: bn_stats/aggr (DVE),
    sqrt + (x-mu)*rstd via activation scale/bias APs (ACT), *gamma
    (GpSimd), +beta (DVE), bf16->fp32 cast on the output DMA (SWDGE).
"""

import os
import sys

import numpy as np

for _p in ("/opt/trn_rl_repo",):
    if _p not in sys.path and os.path.isdir(_p):
        sys.path.insert(0, _p)

import ml_dtypes

import concourse.bass as bass
import concourse.bacc as bacc
import concourse.tile as tile
from concourse import mybir
from concourse.bass_utils import run_bass_kernel_spmd

F32 = mybir.dt.float32
F32R = mybir.dt.float32r
BF16 = mybir.dt.bfloat16
NPBF16 = np.dtype(ml_dtypes.bfloat16)
AF = mybir.ActivationFunctionType
ALU = mybir.AluOpType

# Problem shapes (hardcoded per the spec).
N_COMPANY, N_INDUSTRY, E = 20000, 500, 8192
CC, CI, D, H = 256, 128, 256, 4
HD = D // H  # 64
SCALE = 1.0 / float(np.sqrt(np.float32(HD)))

NCORES = 8
NSH = N_COMPANY // NCORES       # 2500 companies per core
NCT = 20                        # company tiles (19 x 128 + 68)
ITILES = [(t * 128, min(128, N_INDUSTRY - t * 128)) for t in range(4)]

_CACHE = {}
TRACE = False        # set by test.py to request an NTFF profile
LAST_RESULT = None   # BassKernelResults of the most recent run


def _csz(j):
    return min(128, NSH - 128 * j)


def _echunks(ecap):
    return [(c, min(c + 512, ecap)) for c in range(0, ecap, 512)]


DEBUG_TAPS = False


def build_program(etiles_n, pairs):
    """pairs: sorted tuple of (company_tile j, edge_tile t) segment matmuls."""
    E_CAP = etiles_n * 128
    ECH = _echunks(E_CAP)
    nc = bacc.Bacc(debug=False)

    def din(name, shape, dt=BF16):
        return nc.declare_dram_parameter(name, list(shape), dt, isOutput=False)

    cxT = din("cxT", (CC, NSH))            # company_x shard^T
    qxT = din("qxT", (CC, E_CAP))          # company_x rows at unique-src slots
    ixT = din("ixT", (CI, N_INDUSTRY))     # industry_x^T
    WcT = din("WcT", (CC, D))
    WqeT = din("WqeT", (CC, D))            # (wq*scale @ Wc)^T
    WkeT = din("WkeT", (CI, D))            # (wk @ Wi)^T
    WveT = din("WveT", (CI, 4 * 66))       # (wv @ Wi)^T head-padded, 0 at ones col
    woT = din("woT", (D, D))               # w_out^T
    bcr = din("bcr", (1, D))               # bc row
    bor = din("bor", (1, D))               # b_out row
    bvo = din("bvo", (1, 4 * 66))          # bv_eff row w/ 1.0 at ones cols
    gamr = din("gamr", (1, D))
    betr = din("betr", (1, D))
    bqe = din("bqe", (1, D), F32)          # bq_eff (scaled)
    bke = din("bke", (1, D), F32)          # bk_eff
    lncd = din("lncd", (1, 512), F32)      # ln(cnt per industry), padded
    out = nc.declare_dram_parameter("out", [NSH, D], F32, isOutput=True)

    with tile.TileContext(nc) as tc:
        with (
            tc.tile_pool(name="const", bufs=1) as const,
            tc.tile_pool(name="persist", bufs=1) as persist,
            tc.tile_pool(name="work", bufs=3) as work,
            tc.tile_pool(name="ep", bufs=3) as ep,
            tc.tile_pool(name="psA", bufs=4, space="PSUM") as psA,
            tc.tile_pool(name="psC", bufs=2, space="PSUM") as psC,
            tc.tile_pool(name="psX", bufs=2, space="PSUM") as psX,
        ):
            dma = nc.sync.dma_start

            # ---------------- params into SBUF ----------------
            # K/V-side first so the PE can start ASAP.
            def load2(t, rows, cols, tag):
                tiles = []
                for k in range(rows // 128):
                    s = const.tile([128, cols], BF16, name=f"{tag}{k}", tag=f"{tag}{k}")
                    dma(out=s, in_=t[k * 128:(k + 1) * 128, :])
                    tiles.append(s)
                return tiles

            ixT_sb = load2(ixT, CI, N_INDUSTRY, "ixT")[0]
            WkeT_sb = load2(WkeT, CI, D, "WkeT")[0]
            WveT_sb = load2(WveT, CI, 4 * 66, "WveT")[0]

            def row_sb(t, cols, tag):
                s = const.tile([1, cols], BF16, tag=tag)
                dma(out=s, in_=t[:, :])
                return s

            bvo_sb = row_sb(bvo, 4 * 66, "bvo_sb")

            def col_pp(t, tag):
                # [1, 256] f32 DRAM -> [128, 2] SBUF per-partition columns
                s = const.tile([128, 2], F32, tag=tag)
                dma(out=s, in_=bass.AP(tensor=t[:, :].tensor, offset=0,
                                       ap=[[1, 128], [128, 2]]))
                return s

            bk_pp = col_pp(bke, "bk_pp")
            bq_pp = col_pp(bqe, "bq_pp")
            lnc_sb = const.tile([128, 4], F32, tag="lnc_sb")
            dma(out=lnc_sb, in_=bass.AP(tensor=lncd[:, :].tensor, offset=0,
                                        ap=[[1, 128], [128, 4]]))

            qxT_sb = load2(qxT, CC, E_CAP, "qxT")
            WqeT_sb = load2(WqeT, CC, D, "WqeT")
            woT_sb = load2(woT, D, D, "woT")
            bor_sb = row_sb(bor, D, "bor_sb")
            cxT_sb = load2(cxT, CC, NSH, "cxT")
            WcT_sb = load2(WcT, CC, D, "WcT")
            bcr_sb = row_sb(bcr, D, "bcr_sb")

            ohw = din("ohw", (128, max(1, len(pairs)) * 128))
            ohw_sb = const.tile([128, max(1, len(pairs)) * 128], BF16, tag="ohw_sb")
            dma(out=ohw_sb, in_=ohw[:, :])

            def bcast_row(t, tag):
                s = const.tile([128, D], BF16, tag=tag)
                dma(out=s, in_=t[:, :].to_broadcast([128, D]))
                return s

            gam_b = bcast_row(gamr, "gam_b")
            bet_b = bcast_row(betr, "bet_b")

            ones1 = const.tile([1, 128], BF16, name="ones1", tag="ones1")
            nc.vector.memset(ones1, 1.0)

            eps_sb = const.tile([128, 1], F32, name="eps_sb", tag="eps_sb")
            nc.vector.memset(eps_sb, 1e-5)

            def ppbias(colsb, h):
                return colsb[64 * (h % 2):64 * (h % 2) + 64, h // 2:h // 2 + 1]

            # ---------------- K side: khp[h] = [64, 500] bf16 ----------------
            khp = [persist.tile([64, N_INDUSTRY], BF16, name=f"khp{h}", tag=f"khp{h}")
                   for h in range(H)]
            for h in range(H):
                ps = psA.tile([128, 512], F32, name="ps", tag="ps")
                nc.tensor.matmul(ps[0:64, 0:N_INDUSTRY],
                                 WkeT_sb[:, h * 64:(h + 1) * 64],
                                 ixT_sb, start=True, stop=True)
                nc.scalar.activation(khp[h], ps[0:64, 0:N_INDUSTRY],
                                     AF.Identity, bias=ppbias(bk_pp, h), scale=1.0)

            # ---------------- V side: vp[t] = [isz, 4*66] bf16 ---------------
            # col 66h+64 of each head block = 1.0 (softmax denominator row).
            vp = [persist.tile([128, 4 * 66], BF16, name=f"vp{t}", tag=f"vp{t}")
                  for t in range(4)]
            for t, (i0, isz) in enumerate(ITILES):
                ps = psA.tile([128, 512], F32, name="ps", tag="ps")
                nc.tensor.matmul(ps[0:isz, 0:4 * 66],
                                 ixT_sb[:, i0:i0 + isz], WveT_sb,
                                 start=True, stop=False)
                nc.tensor.matmul(ps[0:isz, 0:4 * 66],
                                 ones1[0:1, 0:isz], bvo_sb,
                                 start=False, stop=True)
                nc.scalar.activation(vp[t][0:isz, :], ps[0:isz, 0:4 * 66], AF.Copy)

            # ---------------- Q side: qhp[h] = [64, E_CAP] bf16 --------------
            qhp = [persist.tile([64, E_CAP], BF16, name=f"qhp{h}", tag=f"qhp{h}")
                   for h in range(H)]
            for h in range(H):
                for c0, c1 in ECH:
                    ps = psA.tile([128, 512], F32, name="ps", tag="ps")
                    for k in range(2):
                        nc.tensor.matmul(ps[0:64, 0:c1 - c0],
                                         WqeT_sb[k][:, h * 64:(h + 1) * 64],
                                         qxT_sb[k][:, c0:c1],
                                         start=(k == 0), stop=(k == 1))
                    nc.scalar.activation(qhp[h][0:64, c0:c1], ps[0:64, 0:c1 - c0],
                                         AF.Identity, bias=ppbias(bq_pp, h),
                                         scale=1.0)

            # ------------- attention: scores -> exp -> ctx -> norm -----------
            # ctxT [D, E_CAP] feature-major bf16 (normalized per head)
            ctxT = [persist.tile([128, E_CAP], BF16, name=f"ctxT{d}", tag=f"ctxT{d}")
                    for d in range(2)]
            for h in range(H):
                for c0, c1 in ECH:
                    cw = c1 - c0
                    pc = psC.tile([128, 512], F32, name="pc", tag="pc")
                    for t, (i0, isz) in enumerate(ITILES):
                        ps = psA.tile([128, 512], F32, name="ps", tag="ps")
                        nc.tensor.matmul(ps[0:isz, 0:cw],
                                         khp[h][:, i0:i0 + isz],
                                         qhp[h][:, c0:c1],
                                         start=True, stop=True)
                        pexp = work.tile([128, 512], BF16, name="pexp", tag="pexp")
                        nc.scalar.activation(pexp[0:isz, 0:cw], ps[0:isz, 0:cw],
                                             AF.Exp, bias=lnc_sb[0:isz, t:t + 1],
                                             scale=1.0)
                        nc.tensor.matmul(pc[0:65, 0:cw],
                                         vp[t][0:isz, 66 * h:66 * h + 65],
                                         pexp[0:isz, 0:cw],
                                         start=(t == 0), stop=(t == 3))
                    # normalize rows 0:64 by row 64 (Z): broadcast 1/Z along
                    # partitions via a K=1 f32r matmul, evacuate on ACT, then
                    # multiply (DVE reads one PSUM + one SBUF operand).
                    # custom-DVE ops mis-read PSUM on HW: stage Z in SBUF first
                    zr = work.tile([1, 512], F32, name="zr", tag="zr")
                    nc.scalar.activation(zr[:, 0:cw], pc[64:65, 0:cw], AF.Copy)
                    rd = work.tile([1, 512], F32, name="rd", tag="rd")
                    nc.vector.reciprocal_approx_fast(rd[:, 0:cw], zr[:, 0:cw])
                    rdb = work.tile([1, 512], BF16, name="rdb", tag="rdb")
                    nc.vector.tensor_copy(out=rdb[:, 0:cw], in_=rd[:, 0:cw])
                    pb = psA.tile([128, 512], F32, name="pb", tag="ps")
                    nc.tensor.matmul(pb[0:64, 0:cw], ones1[0:1, 0:64],
                                     rdb[0:1, 0:cw], start=True, stop=True)
                    rb = work.tile([128, 512], BF16, name="rb", tag="rb")
                    nc.scalar.activation(rb[0:64, 0:cw], pb[0:64, 0:cw], AF.Copy)
                    if DEBUG_TAPS and h == 0 and c0 == 0:
                        for nm, tl, dt_ in (("d_rd", rd, F32),
                                            ("d_rdb", rdb, BF16),
                                            ("d_rb", rb, BF16),
                                            ("d_z", zr, F32)):
                            sh = [tl.shape[0], 512]
                            o = nc.declare_dram_parameter(nm, sh, dt_,
                                                          isOutput=True)
                            dma(out=o[:, :], in_=tl[:, :])
                    nc.vector.tensor_tensor(
                        out=ctxT[h // 2][64 * (h % 2):64 * (h % 2) + 64, c0:c1],
                        in0=pc[0:64, 0:cw], in1=rb[0:64, 0:cw], op=ALU.mult)

            # ---------------- attn_out per edge tile (bf16) ------------------
            ao = [persist.tile([128, D], BF16, name=f"ao{t}", tag=f"ao{t}")
                  for t in range(etiles_n)]
            for t in range(etiles_n):
                ps = psA.tile([128, 512], F32, name="ps", tag="ps")
                for k in range(2):
                    nc.tensor.matmul(ps[:, 0:D],
                                     ctxT[k][:, t * 128:(t + 1) * 128],
                                     woT_sb[k], start=(k == 0), stop=False)
                nc.tensor.matmul(ps[:, 0:D], ones1[0:1, 0:128], bor_sb,
                                 start=False, stop=True)
                nc.scalar.activation(ao[t], ps[:, 0:D], AF.Copy)

            if DEBUG_TAPS:
                dbg = {}
                for h in range(H):
                    dbg[f"d_khp{h}"] = khp[h]
                    dbg[f"d_qhp{h}"] = qhp[h]
                for t in range(4):
                    dbg[f"d_vp{t}"] = vp[t]
                for t in range(etiles_n):
                    dbg[f"d_ao{t}"] = ao[t]
                for k in range(2):
                    dbg[f"d_ctxT{k}"] = ctxT[k]
                shapes = {}
                for h in range(H):
                    shapes[f"d_khp{h}"] = (64, N_INDUSTRY)
                    shapes[f"d_qhp{h}"] = (64, E_CAP)
                for t in range(4):
                    shapes[f"d_vp{t}"] = (128, 4 * 66)
                for t in range(etiles_n):
                    shapes[f"d_ao{t}"] = (128, D)
                for k in range(2):
                    shapes[f"d_ctxT{k}"] = (128, E_CAP)
                for name, tl in dbg.items():
                    o = nc.declare_dram_parameter(
                        name, list(shapes[name]), BF16, isOutput=True)
                    dma(out=o[:, :], in_=tl[:, :])

            # ------------- segment sum + residual + layernorm ----------------
            pair_list = list(pairs)
            for j in range(NCT):
                cs = _csz(j)
                jp = [(pi, t) for pi, (jj, t) in enumerate(pair_list) if jj == j]
                px = psX.tile([128, D], F32, name="px", tag="px")
                nc.tensor.matmul(px[0:cs, :], cxT_sb[0][:, 128 * j:128 * j + cs],
                                 WcT_sb[0], start=True, stop=False)
                nc.tensor.matmul(px[0:cs, :], cxT_sb[1][:, 128 * j:128 * j + cs],
                                 WcT_sb[1], start=False, stop=False)
                nc.tensor.matmul(px[0:cs, :], ones1[0:1, 0:cs], bcr_sb,
                                 start=False, stop=(not jp))
                for n, (pi, t) in enumerate(jp):
                    nc.tensor.matmul(px[0:cs, :],
                                     ohw_sb[:, 128 * pi:128 * pi + cs],
                                     ao[t], start=False, stop=(n == len(jp) - 1))
                st = ep.tile([128, nc.vector.BN_STATS_DIM], F32, name="st", tag="st")
                nc.vector.bn_stats(out=st[0:cs, :], in_=px[0:cs, :])
                mv = ep.tile([128, nc.vector.BN_AGGR_DIM], F32, name="mv", tag="mv")
                nc.vector.bn_aggr(out=mv[0:cs, :], in_=st[0:cs, :])
                sd = ep.tile([128, 1], F32, name="sd", tag="sd")
                nc.scalar.activation(sd[0:cs, :], mv[0:cs, 1:2], AF.Sqrt,
                                     bias=eps_sb[0:cs, :], scale=1.0)
                rstd = ep.tile([128, 1], F32, name="rstd", tag="rstd")
                nc.vector.reciprocal(rstd[0:cs, :], sd[0:cs, :])
                nmr = ep.tile([128, 1], F32, name="nmr", tag="nmr")
                nc.vector.tensor_scalar(nmr[0:cs, :], mv[0:cs, 0:1],
                                        rstd[0:cs, 0:1], -1.0,
                                        op0=ALU.mult, op1=ALU.mult)
                xn = ep.tile([128, D], BF16, name="xn", tag="xn")
                nc.scalar.activation(xn[0:cs, :], px[0:cs, :], AF.Identity,
                                     bias=nmr[0:cs, 0:1], scale=rstd[0:cs, 0:1])
                yg = ep.tile([128, D], BF16, name="yg", tag="yg")
                nc.gpsimd.tensor_tensor(out=yg[0:cs, :], in0=xn[0:cs, :],
                                        in1=gam_b[0:cs, :], op=ALU.mult)
                y = ep.tile([128, D], BF16, name="y", tag="y")
                nc.vector.tensor_tensor(out=y[0:cs, :], in0=yg[0:cs, :],
                                        in1=bet_b[0:cs, :], op=ALU.add)
                nc.gpsimd.dma_start(out=out[128 * j:128 * j + cs, :],
                                    in_=y[0:cs, :])

    if not nc.is_finalized():
        nc.finalize()
    return nc


def _bf(a):
    return np.ascontiguousarray(np.asarray(a, np.float32)).astype(NPBF16)


def prepare(company_x, industry_x, edge_index, Wc, bc, Wi, bi,
            w_in, b_in, w_out, b_out, gamma, beta):
    """Host-side prep. Returns (key, shared_map, per_core_maps) or None if the
    input shape assumptions are violated (caller falls back to numpy)."""
    src = np.asarray(edge_index[0], np.int64)
    tgt = np.asarray(edge_index[1], np.int64)
    if src.min() < 0 or src.max() >= N_COMPANY or tgt.min() < 0 \
            or tgt.max() >= N_INDUSTRY:
        return None
    company_x = np.asarray(company_x, np.float32)

    # per-core unique-src slots + (company tile, edge tile) pair set
    cores = []
    for core in range(NCORES):
        lo = core * NSH
        m = (src >= lo) & (src < lo + NSH)
        ls = np.unique(src[m] - lo)
        starts = np.searchsorted(ls, np.arange(NCT) * 128)
        ends = np.searchsorted(ls, np.arange(NCT) * 128 + 128)
        cores.append((lo, ls, starts, ends))

    etiles_n = max(1, max((len(c[1]) + 127) // 128 for c in cores))
    E_CAP = etiles_n * 128
    pairs = set()
    for lo, ls, starts, ends in cores:
        for j in range(NCT):
            if ends[j] > starts[j]:
                for t in range(starts[j] // 128, (ends[j] - 1) // 128 + 1):
                    pairs.add((j, t))
    pairs = tuple(sorted(pairs))
    key = (etiles_n, pairs)

    core_maps = []
    for lo, ls, starts, ends in cores:
        u = len(ls)
        qx = np.zeros((E_CAP, CC), np.float32)
        qx[:u] = company_x[lo + ls]
        ohw = np.zeros((128, len(pairs) * 128), NPBF16)
        for pi, (j, t) in enumerate(pairs):
            s0, s1 = max(starts[j], 128 * t), min(ends[j], 128 * t + 128)
            if s1 > s0:
                sl = np.arange(s0, s1)
                ohw[sl - 128 * t, 128 * pi + (ls[sl] - 128 * j)] = 1.0
        core_maps.append({
            "cxT": _bf(company_x[lo:lo + NSH].T),
            "qxT": _bf(qx.T),
            "ohw": ohw,
        })

    wq, wk, wv = np.split(np.asarray(w_in, np.float32), 3, axis=0)
    bq, bk, bv = np.split(np.asarray(b_in, np.float32), 3)
    Wc = np.asarray(Wc, np.float32); bc = np.asarray(bc, np.float32)
    Wi = np.asarray(Wi, np.float32); bi = np.asarray(bi, np.float32)
    wqs = wq * np.float32(SCALE)
    Wq_eff = wqs @ Wc
    bq_eff = wqs @ bc + bq * np.float32(SCALE)
    Wk_eff = wk @ Wi
    bk_eff = wk @ bi + bk
    Wv_eff = wv @ Wi
    bv_eff = wv @ bi + bv

    WveT = np.zeros((CI, 4 * 66), np.float32)
    bvo = np.zeros((1, 4 * 66), np.float32)
    for h in range(H):
        WveT[:, 66 * h:66 * h + 64] = Wv_eff[64 * h:64 * h + 64, :].T
        bvo[0, 66 * h:66 * h + 64] = bv_eff[64 * h:64 * h + 64]
        bvo[0, 66 * h + 64] = 1.0

    cnt = np.bincount(tgt, minlength=N_INDUSTRY).astype(np.float64)
    lncd = np.full((1, 512), -80.0, np.float32)
    lncd[0, :N_INDUSTRY] = np.log(np.maximum(cnt, 1e-35)).astype(np.float32)

    shared = {
        "ixT": _bf(np.asarray(industry_x, np.float32).T),
        "WcT": _bf(Wc.T),
        "WqeT": _bf(Wq_eff.T),
        "WkeT": _bf(Wk_eff.T),
        "WveT": _bf(WveT),
        "woT": _bf(np.asarray(w_out, np.float32).T),
        "bcr": _bf(bc.reshape(1, D)),
        "bor": _bf(np.asarray(b_out, np.float32).reshape(1, D)),
        "bvo": _bf(bvo),
        "gamr": _bf(np.asarray(gamma, np.float32).reshape(1, D)),
        "betr": _bf(np.asarray(beta, np.float32).reshape(1, D)),
        "bqe": np.ascontiguousarray(bq_eff.reshape(1, D), np.float32),
        "bke": np.ascontiguousarray(bk_eff.reshape(1, D), np.float32),
        "lncd": lncd,
    }
    return key, shared, core_maps


def _numpy_fallback(company_x, industry_x, edge_index, Wc, bc, Wi, bi,
                    w_in, b_in, w_out, b_out, gamma, beta):
    company_h = company_x @ Wc.T + bc
    industry_h = industry_x @ Wi.T + bi
    src, tgt = edge_index[0], edge_index[1]
    e = src.shape[0]
    wq, wk, wv = np.split(w_in, 3, axis=0)
    bq, bk, bv = np.split(b_in, 3)
    qh = (company_h[src] @ wq.T + bq).reshape(e, H, HD)
    kh = (industry_h[tgt] @ wk.T + bk).reshape(e, H, HD)
    vh = (industry_h[tgt] @ wv.T + bv).reshape(e, H, HD)
    scores = np.einsum("qhd,khd->hqk", qh / np.sqrt(HD), kh)
    scores -= scores.max(-1, keepdims=True)
    p = np.exp(scores)
    attn = p / p.sum(-1, keepdims=True)
    ctx = np.einsum("hqk,khd->qhd", attn, vh).reshape(e, D)
    attn_out = ctx @ w_out.T + b_out
    agg = np.zeros((N_COMPANY, D), np.float32)
    np.add.at(agg, src, attn_out)
    counts = np.bincount(src, minlength=N_COMPANY).astype(np.float32)
    pooled = agg / (counts[:, None] + 1e-6)
    out = company_h + pooled
    mean = out.mean(-1, keepdims=True)
    var = out.var(-1, keepdims=True)
    return ((out - mean) / np.sqrt(var + 1e-5) * gamma + beta).astype(np.float32)


def kernel(company_x, industry_x, edge_index, Wc, bc, Wi, bi,
           w_in, b_in, w_out, b_out, gamma, beta):
    args = dict(company_x=np.asarray(company_x, np.float32),
                industry_x=np.asarray(industry_x, np.float32),
                edge_index=np.asarray(edge_index),
                Wc=np.asarray(Wc, np.float32), bc=np.asarray(bc, np.float32),
                Wi=np.asarray(Wi, np.float32), bi=np.asarray(bi, np.float32),
                w_in=np.asarray(w_in, np.float32),
                b_in=np.asarray(b_in, np.float32),
                w_out=np.asarray(w_out, np.float32),
                b_out=np.asarray(b_out, np.float32),
                gamma=np.asarray(gamma, np.float32),
                beta=np.asarray(beta, np.float32))
    prep = prepare(**args)
    if prep is None:
        print("kernel.py: inputs outside compiled assumptions; host fallback",
              file=sys.stderr)
        return _numpy_fallback(**args)
    key, shared, core_maps = prep

    if key not in _CACHE:
        _CACHE[key] = build_program(*key)
    nc = _CACHE[key]

    in_maps = [{**shared, **core_maps[i]} for i in range(NCORES)]
    kw = {}
    if TRACE:
        kw = {"trace": True, "tmpdir": os.environ.get("BASS_TRACE_DIR")}
    res = run_bass_kernel_spmd(nc, in_maps, list(range(NCORES)), **kw)
    global LAST_RESULT
    LAST_RESULT = res
    return np.concatenate([np.asarray(res.results[i]["out"], np.float32)
                           for i in range(NCORES)], axis=0)


# revision 17
# speedup vs baseline: 3.2429x; 1.4282x over previous
"""Trainium2 Bass kernel for CompanyIndustryAttention (gnn_message_passing).

Strategy (all 8 cores, zero collectives):
  - Companies sharded into 8 contiguous ranges of 2500 rows; each edge is
    owned by the core that owns its src company, so the segment-sum scatter
    is core-local (no all-reduce needed).
  - K/V side: tgt indexes only 500 industries, so softmax over the full
    edge set collapses to a count-weighted softmax over the 500 industries
    (exp(s + ln c) = c * exp(s), with ln(c) applied as the per-partition
    activation bias of the Exp).  This turns O(E x E) attention into
    O(E x 500).
  - Q side dedup: edges sharing a src produce identical attention rows, and
    pooled = agg/(cnt+1e-6) ~= attn_out exactly, so only one slot per
    UNIQUE src company is computed (~870/core instead of ~1280 padded).
  - All matmuls run in bf16 (fp32 runs at 1/4 rate on the PE); fp32
    accumulation in PSUM.  Weight chains are pre-fused on the host
    (Wq_eff = wq@Wc etc.) to skip whole matmul stages.
  - The segment-sum one-hot matrices are precomputed on the host and
    DMA'd in as bf16, feeding plain matmuls.
  - Softmax 1/Z broadcast runs on the (otherwise idle) GpSimd engine via
    partition_broadcast; layernorm tail is spread across DVE + ACT.
  - Host inspects the params: all-zero biases / unit gamma / zero beta
    (as in this problem's init) compile to a program with those stages
    elided; the general program is built otherwise.
"""

import os
import sys

import numpy as np

for _p in ("/opt/trn_rl_repo",):
    if _p not in sys.path and os.path.isdir(_p):
        sys.path.insert(0, _p)

import ml_dtypes

import concourse.bass as bass
import concourse.bacc as bacc
import concourse.tile as tile
from concourse import library_config, mybir
from concourse.bass_utils import run_bass_kernel_spmd

F32 = mybir.dt.float32
BF16 = mybir.dt.bfloat16
NPBF16 = np.dtype(ml_dtypes.bfloat16)
AF = mybir.ActivationFunctionType
ALU = mybir.AluOpType

# Problem shapes (hardcoded per the spec).
N_COMPANY, N_INDUSTRY, E = 20000, 500, 8192
CC, CI, D, H = 256, 128, 256, 4
HD = D // H  # 64
SCALE = 1.0 / float(np.sqrt(np.float32(HD)))

NCORES = 8
NSH = N_COMPANY // NCORES       # 2500 companies per core
NCT = 20                        # company tiles (19 x 128 + 68)
ITILES = [(t * 128, min(128, N_INDUSTRY - t * 128)) for t in range(4)]

_CACHE = {}
TRACE = False        # set by test.py to request an NTFF profile
LAST_RESULT = None   # BassKernelResults of the most recent run
DEBUG_TAPS = False
USE_GPB = True       # gpsimd partition_broadcast for the 1/Z broadcast


def _csz(j):
    return min(128, NSH - 128 * j)


def _echunks(ecap):
    return [(c, min(c + 512, ecap)) for c in range(0, ecap, 512)]


def build_program(etiles_n, pairs, flags):
    """pairs: sorted tuple of (company_tile j, edge_tile t) segment matmuls.
    flags: (bc_zero, bo_zero, bv_zero, gamma1_beta0)."""
    bc0, bo0, bv0, g1b0 = flags
    E_CAP = etiles_n * 128
    ECH = _echunks(E_CAP)
    nc = bacc.Bacc(debug=False)

    def din(name, shape, dt=BF16):
        return nc.declare_dram_parameter(name, list(shape), dt, isOutput=False)

    # K/Q-side inputs first so the PE can start ASAP; epilogue-only tensors
    # (cxT, ohw, woT) later — their DMAs overlap the attention phase.
    ixT = din("ixT", (CI, N_INDUSTRY))     # industry_x^T
    WkeT = din("WkeT", (CI, D))            # (wk @ Wi)^T
    WveT = din("WveT", (CI, 4 * 66))       # (wv @ Wi)^T head-padded
    qxT = din("qxT", (CC, E_CAP))          # company_x rows at unique-src slots
    WqeT = din("WqeT", (CC, D))            # (wq*scale @ Wc)^T
    cxT = din("cxT", (CC, NSH))            # company_x shard^T
    WcT = din("WcT", (CC, D))
    woT = din("woT", (D, D))               # w_out^T
    ohw = din("ohw", (128, max(1, len(pairs)) * 128))
    bqe = din("bqe", (1, D), F32)          # bq_eff (scaled)
    bke = din("bke", (1, D), F32)          # bk_eff
    lncd = din("lncd", (1, 512), F32)      # ln(cnt per industry), padded
    if not bv0:
        bvo = din("bvo", (1, 4 * 66))      # bv_eff row w/ 1.0 at ones cols
    if not bc0:
        bcr = din("bcr", (1, D))
    if not bo0:
        bor = din("bor", (1, D))
    if not g1b0:
        gamr = din("gamr", (1, D))
        betr = din("betr", (1, D))
    out = nc.declare_dram_parameter("out", [NSH, D], F32, isOutput=True)

    with tile.TileContext(nc) as tc:
        with (
            tc.tile_pool(name="const", bufs=1) as const,
            tc.tile_pool(name="persist", bufs=1) as persist,
            tc.tile_pool(name="work", bufs=3) as work,
            tc.tile_pool(name="ep", bufs=3) as ep,
            tc.tile_pool(name="psA", bufs=5, space="PSUM") as psA,
            tc.tile_pool(name="psB", bufs=3, space="PSUM") as psB,
        ):
            dma = nc.sync.dma_start

            def load2(t, rows, cols, tag):
                tiles = []
                for k in range(rows // 128):
                    s = const.tile([128, cols], BF16, name=f"{tag}{k}", tag=f"{tag}{k}")
                    dma(out=s, in_=t[k * 128:(k + 1) * 128, :])
                    tiles.append(s)
                return tiles

            def row_sb(t, cols, tag):
                s = const.tile([1, cols], BF16, tag=tag)
                dma(out=s, in_=t[:, :])
                return s

            def col_pp(t, tag):
                # [1, 256] f32 DRAM -> [128, 2] SBUF per-partition columns
                s = const.tile([128, 2], F32, tag=tag)
                dma(out=s, in_=bass.AP(tensor=t[:, :].tensor, offset=0,
                                       ap=[[1, 128], [128, 2]]))
                return s

            ixT_sb = load2(ixT, CI, N_INDUSTRY, "ixT")[0]
            WkeT_sb = load2(WkeT, CI, D, "WkeT")[0]
            WveT_sb = load2(WveT, CI, 4 * 66, "WveT")[0]
            qxT_sb = load2(qxT, CC, E_CAP, "qxT")
            WqeT_sb = load2(WqeT, CC, D, "WqeT")
            bk_pp = col_pp(bke, "bk_pp")
            bq_pp = col_pp(bqe, "bq_pp")
            lnc_sb = const.tile([128, 4], F32, tag="lnc_sb")
            dma(out=lnc_sb, in_=bass.AP(tensor=lncd[:, :].tensor, offset=0,
                                        ap=[[1, 128], [128, 4]]))
            bvo_sb = row_sb(bvo, 4 * 66, "bvo_sb") if not bv0 else None
            cxT_sb = load2(cxT, CC, NSH, "cxT")
            WcT_sb = load2(WcT, CC, D, "WcT")
            woT_sb = load2(woT, D, D, "woT")
            ohw_sb = const.tile([128, max(1, len(pairs)) * 128], BF16, tag="ohw_sb")
            dma(out=ohw_sb, in_=ohw[:, :])
            bcr_sb = row_sb(bcr, D, "bcr_sb") if not bc0 else None
            bor_sb = row_sb(bor, D, "bor_sb") if not bo0 else None

            def bcast_row(t, tag):
                s = const.tile([128, D], BF16, tag=tag)
                dma(out=s, in_=t[:, :].to_broadcast([128, D]))
                return s

            gam_b = bcast_row(gamr, "gam_b") if not g1b0 else None
            bet_b = bcast_row(betr, "bet_b") if not g1b0 else None

            ones1 = const.tile([1, 128], BF16, name="ones1", tag="ones1")
            nc.vector.memset(ones1, 1.0)
            eps_sb = const.tile([128, 1], F32, name="eps_sb", tag="eps_sb")
            nc.vector.memset(eps_sb, 1e-5)

            def ppbias(colsb, h):
                return colsb[64 * (h % 2):64 * (h % 2) + 64, h // 2:h // 2 + 1]

            # ---------------- K side: khp[h] = [64, 500] bf16 ----------------
            khp = [persist.tile([64, N_INDUSTRY], BF16, name=f"khp{h}", tag=f"khp{h}")
                   for h in range(H)]
            for h in range(H):
                ps = psA.tile([128, 512], F32, name="ps", tag="ps")
                nc.tensor.matmul(ps[0:64, 0:N_INDUSTRY],
                                 WkeT_sb[:, h * 64:(h + 1) * 64],
                                 ixT_sb, start=True, stop=True)
                nc.scalar.activation(khp[h], ps[0:64, 0:N_INDUSTRY],
                                     AF.Identity, bias=ppbias(bk_pp, h), scale=1.0)

            # ---------------- V side: vp[t] = [isz, 4, 66] bf16 --------------
            # col 64 of each head block = 1.0 (softmax denominator row).
            vp = [persist.tile([128, 4, 66], BF16, name=f"vp{t}", tag=f"vp{t}")
                  for t in range(4)]
            for t, (i0, isz) in enumerate(ITILES):
                ps = psA.tile([128, 512], F32, name="ps", tag="ps")
                nc.tensor.matmul(ps[0:isz, 0:4 * 66],
                                 ixT_sb[:, i0:i0 + isz], WveT_sb,
                                 start=True, stop=bv0)
                if not bv0:
                    nc.tensor.matmul(ps[0:isz, 0:4 * 66],
                                     ones1[0:1, 0:isz], bvo_sb,
                                     start=False, stop=True)
                nc.vector.tensor_copy(out=vp[t][0:isz, :, :],
                                      in_=ps[0:isz, 0:4 * 66])
                if bv0:
                    nc.vector.memset(vp[t][:, :, 64:65], 1.0)

            # ---------------- Q side: qhp[h] = [64, E_CAP] bf16 --------------
            qhp = [persist.tile([64, E_CAP], BF16, name=f"qhp{h}", tag=f"qhp{h}")
                   for h in range(H)]
            for h in range(H):
                for c0, c1 in ECH:
                    ps = psA.tile([128, 512], F32, name="ps", tag="ps")
                    for k in range(2):
                        nc.tensor.matmul(ps[0:64, 0:c1 - c0],
                                         WqeT_sb[k][:, h * 64:(h + 1) * 64],
                                         qxT_sb[k][:, c0:c1],
                                         start=(k == 0), stop=(k == 1))
                    nc.scalar.activation(qhp[h][0:64, c0:c1], ps[0:64, 0:c1 - c0],
                                         AF.Identity, bias=ppbias(bq_pp, h),
                                         scale=1.0)

            # ------------- attention: scores -> exp -> ctx -> norm -----------
            # ctxT [D, E_CAP] feature-major bf16 (normalized per head)
            ctxT = [persist.tile([128, E_CAP], BF16, name=f"ctxT{d}", tag=f"ctxT{d}")
                    for d in range(2)]
            for h in range(H):
                for c0, c1 in ECH:
                    cw = c1 - c0
                    pc = psB.tile([128, 512], F32, name="pc", tag="pc")
                    for t, (i0, isz) in enumerate(ITILES):
                        ps = psA.tile([128, 512], F32, name="ps", tag="ps")
                        nc.tensor.matmul(ps[0:isz, 0:cw],
                                         khp[h][:, i0:i0 + isz],
                                         qhp[h][:, c0:c1],
                                         start=True, stop=True)
                        pexp = work.tile([128, 512], BF16, name="pexp", tag="pexp")
                        nc.scalar.activation(pexp[0:isz, 0:cw], ps[0:isz, 0:cw],
                                             AF.Exp, bias=lnc_sb[0:isz, t:t + 1],
                                             scale=1.0)
                        nc.tensor.matmul(pc[0:65, 0:cw],
                                         vp[t][0:isz, h, 0:65],
                                         pexp[0:isz, 0:cw],
                                         start=(t == 0), stop=(t == 3))
                    # normalize rows 0:64 by row 64 (Z).  recipfast mis-reads
                    # PSUM on HW, so stage Z into SBUF via ACT first.
                    zr = work.tile([1, 512], F32, name="zr", tag="zr")
                    nc.scalar.activation(zr[:, 0:cw], pc[64:65, 0:cw], AF.Copy)
                    rd = work.tile([1, 512], F32, name="rd", tag="rd")
                    nc.vector.reciprocal_approx_fast(rd[:, 0:cw], zr[:, 0:cw])
                    rb = work.tile([128, 512], F32, name="rb", tag="rb")
                    if USE_GPB:
                        nc.gpsimd.partition_broadcast(rb[0:64, 0:cw],
                                                      rd[0:1, 0:cw],
                                                      channels=64)
                    else:
                        rdb = work.tile([1, 512], BF16, name="rdb", tag="rdb")
                        nc.vector.tensor_copy(out=rdb[:, 0:cw], in_=rd[:, 0:cw])
                        pb = psA.tile([128, 512], F32, name="pb", tag="ps")
                        nc.tensor.matmul(pb[0:64, 0:cw], ones1[0:1, 0:64],
                                         rdb[0:1, 0:cw], start=True, stop=True)
                        nc.scalar.activation(rb[0:64, 0:cw], pb[0:64, 0:cw],
                                             AF.Copy)
                    if DEBUG_TAPS and h == 0 and c0 == 0:
                        for nm, tl, dt_ in (("d_rd", rd, F32), ("d_z", zr, F32),
                                            ("d_rb", rb, F32)):
                            o = nc.declare_dram_parameter(
                                nm, [tl.shape[0], 512], dt_, isOutput=True)
                            dma(out=o[:, :], in_=tl[:, :])
                    nc.vector.tensor_tensor(
                        out=ctxT[h // 2][64 * (h % 2):64 * (h % 2) + 64, c0:c1],
                        in0=pc[0:64, 0:cw], in1=rb[0:64, 0:cw], op=ALU.mult)

            # ---------------- attn_out per edge tile (bf16) ------------------
            ao = [persist.tile([128, D], BF16, name=f"ao{t}", tag=f"ao{t}")
                  for t in range(etiles_n)]
            for t in range(etiles_n):
                ps = psA.tile([128, 512], F32, name="ps", tag="ps")
                nc.tensor.matmul(ps[:, 0:D], ctxT[0][:, t * 128:(t + 1) * 128],
                                 woT_sb[0], start=True, stop=False)
                nc.tensor.matmul(ps[:, 0:D], ctxT[1][:, t * 128:(t + 1) * 128],
                                 woT_sb[1], start=False, stop=bo0)
                if not bo0:
                    nc.tensor.matmul(ps[:, 0:D], ones1[0:1, 0:128], bor_sb,
                                     start=False, stop=True)
                nc.vector.tensor_copy(out=ao[t], in_=ps[:, 0:D])

            if DEBUG_TAPS:
                taps = {}
                for h in range(H):
                    taps[f"d_khp{h}"] = (khp[h], (64, N_INDUSTRY))
                    taps[f"d_qhp{h}"] = (qhp[h], (64, E_CAP))
                for t in range(4):
                    taps[f"d_vp{t}"] = (vp[t], (128, 4 * 66))
                for t in range(etiles_n):
                    taps[f"d_ao{t}"] = (ao[t], (128, D))
                for k in range(2):
                    taps[f"d_ctxT{k}"] = (ctxT[k], (128, E_CAP))
                for name, (tl, sh) in taps.items():
                    o = nc.declare_dram_parameter(name, list(sh), BF16,
                                                  isOutput=True)
                    dma(out=o[:, :], in_=tl[:, :])

            # ------------- segment sum + residual + layernorm ----------------
            pair_list = list(pairs)
            for j in range(NCT):
                cs = _csz(j)
                jp = [(pi, t) for pi, (jj, t) in enumerate(pair_list) if jj == j]
                px = psB.tile([128, 512], F32, name="px", tag="pc")
                nc.tensor.matmul(px[0:cs, 0:D], cxT_sb[0][:, 128 * j:128 * j + cs],
                                 WcT_sb[0], start=True, stop=False)
                nc.tensor.matmul(px[0:cs, 0:D], cxT_sb[1][:, 128 * j:128 * j + cs],
                                 WcT_sb[1], start=False,
                                 stop=(bc0 and not jp))
                if not bc0:
                    nc.tensor.matmul(px[0:cs, 0:D], ones1[0:1, 0:cs], bcr_sb,
                                     start=False, stop=(not jp))
                for n, (pi, t) in enumerate(jp):
                    nc.tensor.matmul(px[0:cs, 0:D],
                                     ohw_sb[:, 128 * pi:128 * pi + cs],
                                     ao[t], start=False, stop=(n == len(jp) - 1))
                st = ep.tile([128, nc.vector.BN_STATS_DIM], F32, name="st", tag="st")
                nc.vector.bn_stats(out=st[0:cs, :], in_=px[0:cs, 0:D])
                mv = ep.tile([128, nc.vector.BN_AGGR_DIM], F32, name="mv", tag="mv")
                nc.vector.bn_aggr(out=mv[0:cs, :], in_=st[0:cs, :])
                sd = ep.tile([128, 1], F32, name="sd", tag="sd")
                nc.scalar.activation(sd[0:cs, :], mv[0:cs, 1:2], AF.Sqrt,
                                     bias=eps_sb[0:cs, :], scale=1.0)
                rstd = ep.tile([128, 1], F32, name="rstd", tag="rstd")
                nc.vector.reciprocal(rstd[0:cs, :], sd[0:cs, :])
                if g1b0:
                    y = ep.tile([128, D], F32, name="y", tag="y")
                    nc.vector.tensor_scalar(y[0:cs, :], px[0:cs, 0:D],
                                            mv[0:cs, 0:1], rstd[0:cs, 0:1],
                                            op0=ALU.subtract, op1=ALU.mult)
                    dma(out=out[128 * j:128 * j + cs, :], in_=y[0:cs, :])
                else:
                    xn = ep.tile([128, D], BF16, name="xn", tag="xn")
                    nc.vector.tensor_scalar(xn[0:cs, :], px[0:cs, 0:D],
                                            mv[0:cs, 0:1], rstd[0:cs, 0:1],
                                            op0=ALU.subtract, op1=ALU.mult)
                    yg = ep.tile([128, D], BF16, name="yg", tag="yg")
                    nc.gpsimd.tensor_tensor(out=yg[0:cs, :], in0=xn[0:cs, :],
                                            in1=gam_b[0:cs, :], op=ALU.mult)
                    y = ep.tile([128, D], F32, name="y", tag="y")
                    nc.vector.tensor_tensor(out=y[0:cs, :], in0=yg[0:cs, :],
                                            in1=bet_b[0:cs, :], op=ALU.add)
                    dma(out=out[128 * j:128 * j + cs, :], in_=y[0:cs, :])

    if not nc.is_finalized():
        nc.finalize()
    return nc


def _bf(a):
    return np.ascontiguousarray(np.asarray(a, np.float32)).astype(NPBF16)


def prepare(company_x, industry_x, edge_index, Wc, bc, Wi, bi,
            w_in, b_in, w_out, b_out, gamma, beta):
    """Host-side prep. Returns (key, shared_map, per_core_maps) or None if the
    input shape assumptions are violated (caller falls back to numpy)."""
    src = np.asarray(edge_index[0], np.int64)
    tgt = np.asarray(edge_index[1], np.int64)
    if src.min() < 0 or src.max() >= N_COMPANY or tgt.min() < 0 \
            or tgt.max() >= N_INDUSTRY:
        return None
    company_x = np.asarray(company_x, np.float32)

    # per-core unique-src slots + (company tile, edge tile) pair set
    cores = []
    for core in range(NCORES):
        lo = core * NSH
        m = (src >= lo) & (src < lo + NSH)
        ls = np.unique(src[m] - lo)
        starts = np.searchsorted(ls, np.arange(NCT) * 128)
        ends = np.searchsorted(ls, np.arange(NCT) * 128 + 128)
        cores.append((lo, ls, starts, ends))

    etiles_n = max(1, max((len(c[1]) + 127) // 128 for c in cores))
    E_CAP = etiles_n * 128
    pairs = set()
    for lo, ls, starts, ends in cores:
        for j in range(NCT):
            if ends[j] > starts[j]:
                for t in range(starts[j] // 128, (ends[j] - 1) // 128 + 1):
                    pairs.add((j, t))
    pairs = tuple(sorted(pairs))

    wq, wk, wv = np.split(np.asarray(w_in, np.float32), 3, axis=0)
    bq, bk, bv = np.split(np.asarray(b_in, np.float32), 3)
    Wc = np.asarray(Wc, np.float32); bc = np.asarray(bc, np.float32)
    Wi = np.asarray(Wi, np.float32); bi = np.asarray(bi, np.float32)
    b_out = np.asarray(b_out, np.float32)
    gamma = np.asarray(gamma, np.float32)
    beta = np.asarray(beta, np.float32)
    wqs = wq * np.float32(SCALE)
    Wq_eff = wqs @ Wc
    bq_eff = wqs @ bc + bq * np.float32(SCALE)
    Wk_eff = wk @ Wi
    bk_eff = wk @ bi + bk
    Wv_eff = wv @ Wi
    bv_eff = wv @ bi + bv

    flags = (not bc.any(), not b_out.any(), not bv_eff.any(),
             bool(np.all(gamma == 1.0) and not beta.any()))
    key = (etiles_n, pairs, flags)

    core_maps = []
    for lo, ls, starts, ends in cores:
        u = len(ls)
        qx = np.zeros((E_CAP, CC), np.float32)
        qx[:u] = company_x[lo + ls]
        ohw = np.zeros((128, len(pairs) * 128), NPBF16)
        for pi, (j, t) in enumerate(pairs):
            s0, s1 = max(starts[j], 128 * t), min(ends[j], 128 * t + 128)
            if s1 > s0:
                sl = np.arange(s0, s1)
                ohw[sl - 128 * t, 128 * pi + (ls[sl] - 128 * j)] = 1.0
        core_maps.append({
            "cxT": _bf(company_x[lo:lo + NSH].T),
            "qxT": _bf(qx.T),
            "ohw": ohw,
        })

    WveT = np.zeros((CI, 4 * 66), np.float32)
    bvo = np.zeros((1, 4 * 66), np.float32)
    for h in range(H):
        WveT[:, 66 * h:66 * h + 64] = Wv_eff[64 * h:64 * h + 64, :].T
        bvo[0, 66 * h:66 * h + 64] = bv_eff[64 * h:64 * h + 64]
        bvo[0, 66 * h + 64] = 1.0

    cnt = np.bincount(tgt, minlength=N_INDUSTRY).astype(np.float64)
    lncd = np.full((1, 512), -80.0, np.float32)
    lncd[0, :N_INDUSTRY] = np.log(np.maximum(cnt, 1e-35)).astype(np.float32)

    shared = {
        "ixT": _bf(np.asarray(industry_x, np.float32).T),
        "WcT": _bf(Wc.T),
        "WqeT": _bf(Wq_eff.T),
        "WkeT": _bf(Wk_eff.T),
        "WveT": _bf(WveT),
        "woT": _bf(np.asarray(w_out, np.float32).T),
        "bqe": np.ascontiguousarray(bq_eff.reshape(1, D), np.float32),
        "bke": np.ascontiguousarray(bk_eff.reshape(1, D), np.float32),
        "lncd": lncd,
    }
    bc0, bo0, bv0, g1b0 = flags
    if not bc0:
        shared["bcr"] = _bf(bc.reshape(1, D))
    if not bo0:
        shared["bor"] = _bf(b_out.reshape(1, D))
    if not bv0:
        shared["bvo"] = _bf(bvo)
    if not g1b0:
        shared["gamr"] = _bf(gamma.reshape(1, D))
        shared["betr"] = _bf(beta.reshape(1, D))
    return key, shared, core_maps


def _numpy_fallback(company_x, industry_x, edge_index, Wc, bc, Wi, bi,
                    w_in, b_in, w_out, b_out, gamma, beta):
    company_h = company_x @ Wc.T + bc
    industry_h = industry_x @ Wi.T + bi
    src, tgt = edge_index[0], edge_index[1]
    e = src.shape[0]
    wq, wk, wv = np.split(w_in, 3, axis=0)
    bq, bk, bv = np.split(b_in, 3)
    qh = (company_h[src] @ wq.T + bq).reshape(e, H, HD)
    kh = (industry_h[tgt] @ wk.T + bk).reshape(e, H, HD)
    vh = (industry_h[tgt] @ wv.T + bv).reshape(e, H, HD)
    scores = np.einsum("qhd,khd->hqk", qh / np.sqrt(HD), kh)
    scores -= scores.max(-1, keepdims=True)
    p = np.exp(scores)
    attn = p / p.sum(-1, keepdims=True)
    ctx = np.einsum("hqk,khd->qhd", attn, vh).reshape(e, D)
    attn_out = ctx @ w_out.T + b_out
    agg = np.zeros((N_COMPANY, D), np.float32)
    np.add.at(agg, src, attn_out)
    counts = np.bincount(src, minlength=N_COMPANY).astype(np.float32)
    pooled = agg / (counts[:, None] + 1e-6)
    out = company_h + pooled
    mean = out.mean(-1, keepdims=True)
    var = out.var(-1, keepdims=True)
    return ((out - mean) / np.sqrt(var + 1e-5) * gamma + beta).astype(np.float32)


def kernel(company_x, industry_x, edge_index, Wc, bc, Wi, bi,
           w_in, b_in, w_out, b_out, gamma, beta):
    args = dict(company_x=np.asarray(company_x, np.float32),
                industry_x=np.asarray(industry_x, np.float32),
                edge_index=np.asarray(edge_index),
                Wc=np.asarray(Wc, np.float32), bc=np.asarray(bc, np.float32),
                Wi=np.asarray(Wi, np.float32), bi=np.asarray(bi, np.float32),
                w_in=np.asarray(w_in, np.float32),
                b_in=np.asarray(b_in, np.float32),
                w_out=np.asarray(w_out, np.float32),
                b_out=np.asarray(b_out, np.float32),
                gamma=np.asarray(gamma, np.float32),
                beta=np.asarray(beta, np.float32))
    prep = prepare(**args)
    if prep is None:
        print("kernel.py: inputs outside compiled assumptions; host fallback",
              file=sys.stderr)
        return _numpy_fallback(**args)
    key, shared, core_maps = prep

    if key not in _CACHE:
        _CACHE[key] = build_program(*key)
    nc = _CACHE[key]

    in_maps = [{**shared, **core_maps[i]} for i in range(NCORES)]
    kw = {}
    if TRACE:
        kw = {"trace": True, "tmpdir": os.environ.get("BASS_TRACE_DIR")}
    res = run_bass_kernel_spmd(nc, in_maps, list(range(NCORES)), **kw)
    global LAST_RESULT
    LAST_RESULT = res
    return np.concatenate([np.asarray(res.results[i]["out"], np.float32)
                           for i in range(NCORES)], axis=0)
